# revision 7
# baseline (speedup 1.0000x reference)
"""DiffVolumeV2 Trainium2 kernel.

out[b,c,d,h,x] = left[b,c,h,x] - right[b,c,h, clip(4x - d + 1, 0, Wr-1)]
with B=4, C=32, H=80, Wl=160, Wr=640, D=48.

Every (b,c,h) row is independent, so the 10240 rows are sharded
contiguously across the 8 NeuronCores (1280 rows/core, 10 tiles of 128
partitions each).

The gather collapses into plain affine access patterns: writing d = 4q+s
(s in 0..3, q in 0..11) gives index 4x+1-d = (4x) + (1-s) + (-4q), which is
affine in (s, q, x).  Negative indices (the clip-to-0 region) are handled by
prepending a 46-element pad in front of the right-row tile, pre-filled with
right[row, 0] (exactly the clipped value).  One tensor_sub per 128-row tile
then produces all 48 disparities at once.
"""

import numpy as np
from concourse import bacc, bass, tile
from concourse.bass_utils import run_bass_kernel_spmd
import concourse.mybir as mybir

B, C, H, WL, WR, D = 4, 32, 80, 160, 640, 48
N_CORES = 8
R = B * C * H            # 10240 independent rows
RPC = R // N_CORES       # 1280 rows per core
P = 128                  # SBUF partitions
TILES = RPC // P         # 10 tiles per core
PAD = 46                 # max negative reach of 4x+1-d (d=47, x=0 -> -46)

_cached = None


RTW = PAD + WR   # 686-element per-tile slot in the right SBUF buffer


def _build() -> bass.Bass:
    # Bacc (not raw Bass): its compile() pipeline runs
    # generate_event_semaphores, which splits multi-sem sync waits to satisfy
    # the 1-wait-per-instruction TRN2 ISA limit.
    nc = bacc.Bacc()
    left_p = nc.declare_dram_parameter("left", [RPC, WL], mybir.dt.float32, isOutput=False)
    right_p = nc.declare_dram_parameter("right", [RPC, WR], mybir.dt.float32, isOutput=False)
    out_p = nc.declare_dram_parameter("out", [RPC, D, WL], mybir.dt.float32, isOutput=True)
    out_flat = out_p[:].rearrange("r d x -> r (d x)")
    T = TILES

    with tile.TileContext(nc) as tc:
        with tc.tile_pool(name="inp", bufs=1) as inp_pool, \
             tc.tile_pool(name="ot", bufs=2) as ot_pool:
            rt = inp_pool.tile([P, T * RTW], mybir.dt.float32)
            lt = inp_pool.tile([P, T * WL], mybir.dt.float32)

            # Preload ALL input rows in one DMA per tensor.  SBUF free-dim
            # index (t, x) maps to DRAM row t*128 + p.
            rt_dst = bass.AP(rt.tensor, rt.offset + PAD, [list(rt.ap[0]), [RTW, T], [1, WR]])
            rt_src = bass.AP(right_p[:].tensor, 0, [[WR, P], [WR * P, T], [1, WR]])
            nc.sync.dma_start(out=rt_dst, in_=rt_src)
            lt_dst = bass.AP(lt.tensor, lt.offset, [list(lt.ap[0]), [WL, T], [1, WL]])
            lt_src = bass.AP(left_p[:].tensor, 0, [[WL, P], [WL * P, T], [1, WL]])
            nc.sync.dma_start(out=lt_dst, in_=lt_src)

            # Fill every tile's pad region with right[row, 0] (the clip value).
            # in1 reads lt with a bypass op purely so this instruction absorbs
            # BOTH input-DMA waits; the SUBs below then inherit the deps via
            # same-engine program order and carry at most one sync wait each
            # (walrus TT lowers to the packed S3S3D3_TT struct with almost no
            # sync-wait room; scalar_tensor_tensor lowers to S2S2D2_STT which
            # has more).
            fill_out = bass.AP(rt.tensor, rt.offset, [list(rt.ap[0]), [RTW, T], [1, PAD]])
            fill_in0 = bass.AP(rt.tensor, rt.offset + PAD, [list(rt.ap[0]), [RTW, T], [0, PAD]])
            fill_in1 = bass.AP(lt.tensor, lt.offset, [list(lt.ap[0]), [0, T], [0, PAD]])
            nc.vector.scalar_tensor_tensor(fill_out, fill_in0, 0.0, fill_in1,
                                           op0=mybir.AluOpType.bypass,
                                           op1=mybir.AluOpType.bypass)

            for t in range(T):
                ot = ot_pool.tile([P, D * WL], mybir.dt.float32, name=f"ot{t}", tag="ot")
                # out[d*WL + x] = left[x] - rt[PAD + 1 - d + 4x]: 2-D (d, x)
                # APs keep walrus in the S2S2D2_TT struct, which still has
                # room for its sync-wait commands (3-D operands do not).
                lt_v = bass.AP(lt.tensor, lt.offset + t * WL,
                               [list(lt.ap[0]), [0, D], [1, WL]])
                rt_v = bass.AP(rt.tensor, rt.offset + t * RTW + PAD + 1,
                               [list(rt.ap[0]), [-1, D], [4, WL]])
                nc.vector.scalar_tensor_tensor(ot[:, :], lt_v, 0.0, rt_v,
                                               op0=mybir.AluOpType.bypass,
                                               op1=mybir.AluOpType.subtract)

                nc.scalar.dma_start(out=out_flat[t * P:(t + 1) * P, :], in_=ot[:, :])

    # Run Bacc's compile pipeline (register allocation, event-semaphore wait
    # splitting, ...) — the axon/pjrt exec path does not call finalize itself.
    nc.finalize()
    return nc


def _run(left_feature, right_feature, trace=False, **trace_kw):
    global _cached
    left = np.ascontiguousarray(np.asarray(left_feature, dtype=np.float32).reshape(R, WL))
    right = np.ascontiguousarray(np.asarray(right_feature, dtype=np.float32).reshape(R, WR))
    if _cached is None:
        _cached = _build()
    nc = _cached
    in_maps = [
        {"left": left[i * RPC:(i + 1) * RPC], "right": right[i * RPC:(i + 1) * RPC]}
        for i in range(N_CORES)
    ]
    res = run_bass_kernel_spmd(nc, in_maps, list(range(N_CORES)), trace=trace, **trace_kw)
    shards = [res.results[i]["out"] for i in range(N_CORES)]
    full = np.concatenate(shards, axis=0).reshape(B, C, H, D, WL).transpose(0, 1, 3, 2, 4)
    return np.ascontiguousarray(full), res


def kernel(left_feature, right_feature, max_disp=48, **_ignored):
    assert int(max_disp) == D
    out, _ = _run(left_feature, right_feature, trace=False)
    return out
